# revision 1
# baseline (speedup 1.0000x reference)
"""Multi-head attention Trainium2 Bass kernel.

Problem: B=4, N=M=2048, DM=512, H=8, DH=64, DO=512, fp32.
Sharding: 8 cores = (batch b, row-half) -- each core computes full attention
for 1024 query rows of one batch. No collectives.

Per-core dataflow (matmuls in fp32r, 1 cycle/row):
  - PE-transpose Q,K,V 128x128 blocks (streamed, never fully resident)
  - qTf = Wq_flat.T @ QT   [hdh, n]   (bias + 1/sqrt(dh) folded host-side)
  - kTf = Wk_flat.T @ KT   [hdh, m]
  - vh  = VT.T @ Wv_flat   [m, hdh]   (+ ones column per head for softmax sums)
  - scoresT[m,n] = kh @ qhT  (2 heads row-packed in PE via tile_position)
  - exp on ScalarE (PSUM->SBUF fp32r); no max subtraction (|logits| < ~12)
  - ohT[dh+1, n] = [vh|1].T @ expT  (row dh = softmax denominator)
  - normalize (reciprocal + PE ones-broadcast) + v-bias -> mh_h [dh, n]
  - out[n, do] = sum_h mh_h.T @ Wp_h + bias (ones-row matmul)
"""
import os
import sys

sys.path.insert(0, "/opt/trn_rl_repo")

import numpy as np

import concourse.bass as bass
import concourse.mybir as mybir
import concourse.tile as tile
from concourse import bacc
from concourse.bass_utils import run_bass_kernel_spmd

F32 = mybir.dt.float32
F32R = mybir.dt.float32r
EXP = mybir.ActivationFunctionType.Exp
ADD = mybir.AluOpType.add
MULT = mybir.AluOpType.mult

P = 128
DM = 512
HDH = 512
DH = 64
H = 8
NB = 1024     # query rows per core
M = 2048      # kv rows
DO = 512
N_MT = M // P
N_QT = NB // P

_CACHED = {}
LAST_EXEC_NS = None


def _build():
    nc = bacc.Bacc("TRN2", target_bir_lowering=False, debug=False)

    d_q = nc.declare_dram_parameter("q", [NB, DM], F32, isOutput=False)
    d_k = nc.declare_dram_parameter("k", [M, DM], F32, isOutput=False)
    d_v = nc.declare_dram_parameter("v", [M, DM], F32, isOutput=False)
    d_wq = nc.declare_dram_parameter("wq", [DM, HDH], F32R, isOutput=False)
    d_wk = nc.declare_dram_parameter("wk", [DM, HDH], F32R, isOutput=False)
    d_wv = nc.declare_dram_parameter("wv", [DM, HDH], F32R, isOutput=False)
    d_wp = nc.declare_dram_parameter("wp", [HDH, DO], F32R, isOutput=False)
    d_qb = nc.declare_dram_parameter("qb", [P, 4], F32, isOutput=False)
    d_kb = nc.declare_dram_parameter("kb", [P, 4], F32, isOutput=False)
    d_vb = nc.declare_dram_parameter("vb", [DH, H], F32, isOutput=False)
    d_pb = nc.declare_dram_parameter("pb", [1, DO], F32R, isOutput=False)
    d_id = nc.declare_dram_parameter("ident", [P, P], F32, isOutput=False)
    d_ones = nc.declare_dram_parameter("ones", [P, P], F32R, isOutput=False)
    d_out = nc.declare_dram_parameter("out", [NB, DO], F32, isOutput=True)

    with tile.TileContext(nc) as tc:
        from contextlib import ExitStack
        with ExitStack() as ctx:
            persist = ctx.enter_context(tc.tile_pool(name="persist", bufs=1))
            aw = ctx.enter_context(tc.tile_pool(name="aw", bufs=1))
            raw = ctx.enter_context(tc.tile_pool(name="raw", bufs=5))
            kts_pool = ctx.enter_context(tc.tile_pool(name="kts", bufs=2))
            exp_pool = ctx.enter_context(tc.tile_pool(name="expp", bufs=5))
            nm = ctx.enter_context(tc.tile_pool(name="nm", bufs=1))
            ps = ctx.enter_context(tc.tile_pool(name="ps", bufs=2, space="PSUM"))
            ps_sc = ctx.enter_context(tc.tile_pool(name="ps_sc", bufs=2, space="PSUM"))
            ps_oh = ctx.enter_context(tc.tile_pool(name="ps_oh", bufs=1, space="PSUM"))

            # --- constants ---
            ident = persist.tile([P, P], F32R, tag="ident", name="ident")
            nc.gpsimd.dma_start(ident[:], d_id[:].bitcast(F32R))
            qb = persist.tile([P, 4], F32, tag="qb", name="qb")
            nc.gpsimd.dma_start(qb[:], d_qb[:])
            kb = persist.tile([P, 4], F32, tag="kb", name="kb")
            nc.gpsimd.dma_start(kb[:], d_kb[:])
            # non-critical loads are deferred so the first K/Q DMAs go out first
            ones = persist.tile([P, P], F32R, tag="ones", name="ones")
            vb = persist.tile([DH, H], F32, tag="vb", name="vb")
            pb = persist.tile([1, DO], F32R, tag="pb", name="pb")
            wp_sb = [persist.tile([DH, DO], F32R, tag=f"wp{h}", name=f"wp{h}")
                     for h in range(H)]

            # --- persistent activations ---
            kTf = [persist.tile([P, M], F32R, tag=f"kTf{i}", name=f"kTf{i}")
                   for i in range(4)]
            qTf = [persist.tile([P, NB], F32R, tag=f"qTf{i}", name=f"qTf{i}")
                   for i in range(4)]
            vha = persist.tile([P, N_MT, 8 * 65], F32R, tag="vha", name="vha")
            mh = [persist.tile([DH, NB], F32R, tag=f"mh{h}", name=f"mh{h}")
                  for h in range(H)]

            def load_w(d_w):
                w_sb = []
                for dc in range(4):
                    t = aw.tile([P, HDH], F32R, tag=f"w{dc}", name=f"w{dc}")
                    nc.gpsimd.dma_start(t[:], d_w[dc * P:(dc + 1) * P, :])
                    w_sb.append(t)
                return w_sb

            def transpose_group(d_src, t0, n_tiles, tag):
                """Transpose rows [t0*P, (t0+n_tiles)*P) of d_src into an
                SBUF tile [P, 4, n_tiles*P] ([dm-chunk, dc, row])."""
                ts = kts_pool.tile([P, 4, 512], F32R, tag="kts", name="kts")
                for j in range(n_tiles):
                    rn = raw.tile([P, DM], F32R, tag="araw", name="araw")
                    nc.sync.dma_start(
                        rn[:], d_src[(t0 + j) * P:(t0 + j + 1) * P, :].bitcast(F32R))
                    pst = ps.tile([P, DM], F32R, tag="pj", name="pj")
                    for dc in range(4):
                        nc.tensor.transpose(
                            pst[:, dc * P:(dc + 1) * P], rn[:, dc * P:(dc + 1) * P],
                            ident[:],
                        )
                    nc.scalar.copy(
                        ts[:, :, j * P:(j + 1) * P],
                        pst.rearrange("p (a b) -> p a b", a=4),
                    )
                return ts

            # === Phase A: K then Q (transposed projections), then V ===
            wk_sb = load_w(d_wk)
            for ms in range(4):
                ts = transpose_group(d_k, ms * 4, 4, "k")
                for ht in range(4):
                    pp = ps.tile([P, 512], F32, tag="pj", name="pj")
                    for dc in range(4):
                        nc.tensor.matmul(
                            pp[:], wk_sb[dc][:, ht * P:(ht + 1) * P], ts[:, dc, :],
                            start=(dc == 0), stop=(dc == 3),
                        )
                    nc.vector.tensor_scalar(
                        kTf[ht][:, ms * 512:(ms + 1) * 512],
                        pp[:], kb[:, ht:ht + 1], None, ADD,
                    )
            wq_sb = load_w(d_wq)
            for ns in range(2):
                ts = transpose_group(d_q, ns * 4, 4, "q")
                for ht in range(4):
                    pp = ps.tile([P, 512], F32, tag="pj", name="pj")
                    for dc in range(4):
                        nc.tensor.matmul(
                            pp[:], wq_sb[dc][:, ht * P:(ht + 1) * P], ts[:, dc, :],
                            start=(dc == 0), stop=(dc == 3),
                        )
                    nc.vector.tensor_scalar(
                        qTf[ht][:, ns * 512:(ns + 1) * 512],
                        pp[:], qb[:, ht:ht + 1], None, ADD,
                    )
            wv_sb = load_w(d_wv)
            # ones columns of vh_aug (col 64 of each head group)
            nc.gpsimd.dma_start(ones[:], d_ones[:])
            nc.vector.tensor_copy(
                vha.rearrange("p a (h c) -> p a h c", c=65)[:, :, :, 64:65],
                ones[:, 0:N_MT * 8].bitcast(F32).rearrange(
                    "p (a h) -> p a h", a=N_MT)[:, :, :, None],
            )
            nc.gpsimd.dma_start(vb[:], d_vb[:])

            def emit_v_tile(mt):
                """Transpose + project one V m-tile into vh_aug (streamed,
                interleaved with the first attention pair so the PE keeps the
                ScalarE exp pipeline fed from the start)."""
                vn = raw.tile([P, DM], F32R, tag="araw", name="araw")
                nc.sync.dma_start(
                    vn[:], d_v[mt * P:(mt + 1) * P, :].bitcast(F32R))
                pst = ps.tile([P, DM], F32R, tag="pj", name="pj")
                for dc in range(4):
                    nc.tensor.transpose(
                        pst[:, dc * P:(dc + 1) * P], vn[:, dc * P:(dc + 1) * P],
                        ident[:],
                    )
                vtt = raw.tile([P, 4, P], F32R, tag="vtt", name="vtt")
                nc.vector.tensor_copy(vtt[:], pst.rearrange("p (a b) -> p a b", a=4))
                pp = ps.tile([P, 512], F32, tag="pj", name="pj")
                for dc in range(4):
                    nc.tensor.matmul(
                        pp[:], vtt[:, dc, :], wv_sb[dc][:],
                        start=(dc == 0), stop=(dc == 3),
                    )
                nc.vector.tensor_copy(
                    vha.rearrange("p a (h c) -> p a h c", c=65)[:, mt, :, 0:64],
                    pp.rearrange("p (h c) -> p h c", h=H),
                )

            def emit_out_group(nt):
                po = ps.tile([P, DO], F32, tag="pj", name="pj")
                for h in range(H):
                    nc.tensor.matmul(
                        po[:], mh[h][:, nt * P:(nt + 1) * P], wp_sb[h][:],
                        start=(h == 0), stop=False, skip_group_check=True,
                    )
                nc.tensor.matmul(
                    po[:], ones[0:1, 0:P], pb[:],
                    start=False, stop=True, skip_group_check=True,
                )
                ot = nm.tile([P, DO], F32, tag="rc", name="rc")
                nc.vector.tensor_copy(ot[:], po[:])
                nc.gpsimd.dma_start(d_out[nt * P:(nt + 1) * P, :], ot[:])

            # === Phase B: attention (nb outer so phase C can interleave) ===
            for h in range(H):
                nc.gpsimd.dma_start(wp_sb[h][:], d_wp[h * DH:(h + 1) * DH, :])
            nc.gpsimd.dma_start(pb[:], d_pb[:])
            for nb in range(2):
                for hp in range(4):
                    ns = slice(nb * 512, (nb + 1) * 512)
                    oh = {}
                    for ab in range(2):
                        oh[ab] = ps_oh.tile([P, 512], F32, tag=f"oh{ab}",
                                            name=f"oh{ab}")
                    for mu in range(N_MT // 2):
                        if nb == 0 and hp == 0:
                            emit_v_tile(2 * mu)
                            emit_v_tile(2 * mu + 1)
                        ex = {}
                        for ab in range(2):
                            base = ab * 64
                            sc = ps_sc.tile([P, 1024], F32, tag="sc", name="sc")
                            for j in range(2):
                                mt = 2 * mu + j
                                nc.tensor.matmul(
                                    sc[:, j * 512:(j + 1) * 512],
                                    kTf[hp][base:base + 64, mt * P:(mt + 1) * P],
                                    qTf[hp][base:base + 64, ns],
                                    start=True, stop=True,
                                    tile_position=(base, 0),
                                )
                            ex[ab] = exp_pool.tile([P, 1024], F32R, tag="ex",
                                                   name="ex")
                            nc.scalar.activation(ex[ab][:], sc[:], EXP)
                        for ab in range(2):
                            h = 2 * hp + ab
                            for j in range(2):
                                mt = 2 * mu + j
                                nc.tensor.matmul(
                                    oh[ab][0:65, :],
                                    vha[:, mt, h * 65:h * 65 + 65],
                                    ex[ab][:, j * 512:(j + 1) * 512],
                                    start=(mu == 0 and j == 0),
                                    stop=(mu == N_MT // 2 - 1 and j == 1),
                                )
                    # normalization + v-bias
                    for ab in range(2):
                        h = 2 * hp + ab
                        sums_sb = nm.tile([P, 512], F32, tag="sums", name="sums")
                        nc.vector.tensor_copy(sums_sb[64:65, :], oh[ab][64:65, :])
                        from concourse.dve_ops import (
                            RECIP_APPROX_FAST_CONSTS, RECIPROCAL_APPROX_FAST)
                        _c = RECIP_APPROX_FAST_CONSTS
                        rr = nm.tile([P, 512], F32R, tag="rr", name="rr")
                        nc.vector._custom_dve(
                            RECIPROCAL_APPROX_FAST, out=rr[:], in0=sums_sb[:],
                            s0=_c["s0"], s1=_c["s1"], imm2=_c["imm2"],
                        )
                        bc_ps = ps.tile([64, 512], F32, tag="pj", name="pj")
                        nc.tensor.matmul(
                            bc_ps[:], ones[64:65, 0:64], rr[64:65, :],
                            start=True, stop=True,
                        )
                        bc_sb = nm.tile([64, 512], F32, tag="bcs", name="bcs")
                        nc.vector.tensor_copy(bc_sb[:], bc_ps[:])
                        tmp = nm.tile([64, 512], F32, tag="tmp", name="tmp")
                        nc.vector.tensor_tensor(tmp[:], oh[ab][0:64, :], bc_sb[:],
                                                MULT)
                        nc.vector.tensor_scalar(
                            mh[h][:, ns], tmp[:], vb[:, h:h + 1], None, ADD,
                        )
                    if nb == 1:
                        # fill ScalarE-bound window with nb0's output projection
                        emit_out_group(hp)

            # === Phase C tail: second n-half output projection ===
            for nt in range(4, N_QT):
                emit_out_group(nt)

    nc.compile()
    return nc


def kernel(query, key, value, query_kernel, key_kernel, value_kernel,
           projection_kernel, q_bias, k_bias, v_bias, projection_bias):
    query = np.ascontiguousarray(np.asarray(query, dtype=np.float32))
    key = np.ascontiguousarray(np.asarray(key, dtype=np.float32))
    value = np.ascontiguousarray(np.asarray(value, dtype=np.float32))
    scale = np.float32(1.0 / 8.0)  # 1/sqrt(DH)

    wq = np.ascontiguousarray(
        (np.asarray(query_kernel, np.float32) * scale).transpose(1, 0, 2).reshape(DM, HDH))
    wk = np.ascontiguousarray(
        np.asarray(key_kernel, np.float32).transpose(1, 0, 2).reshape(DM, HDH))
    wv = np.ascontiguousarray(
        np.asarray(value_kernel, np.float32).transpose(1, 0, 2).reshape(DM, HDH))
    wp = np.ascontiguousarray(np.asarray(projection_kernel, np.float32).reshape(HDH, DO))
    qb = np.ascontiguousarray(
        (np.asarray(q_bias, np.float32) * scale).reshape(HDH).reshape(4, P).T)
    kb = np.ascontiguousarray(np.asarray(k_bias, np.float32).reshape(HDH).reshape(4, P).T)
    vb = np.ascontiguousarray(np.asarray(v_bias, np.float32).reshape(H, DH).T)
    pb = np.ascontiguousarray(np.asarray(projection_bias, np.float32).reshape(1, DO))
    ident = np.eye(P, dtype=np.float32)
    ones = np.ones((P, P), dtype=np.float32)

    if "nc" not in _CACHED:
        _CACHED["nc"] = _build()
    nc = _CACHED["nc"]

    shared = dict(wq=wq, wk=wk, wv=wv, wp=wp, qb=qb, kb=kb, vb=vb, pb=pb,
                  ident=ident, ones=ones)
    in_maps = []
    for c in range(8):
        b, half = c // 2, c % 2
        in_maps.append(dict(
            q=np.ascontiguousarray(query[b, half * NB:(half + 1) * NB, :]),
            k=key[b], v=value[b], **shared))

    trace = os.environ.get("KERNEL_TRACE", "0") == "1"
    try:
        res = run_bass_kernel_spmd(nc, in_maps, core_ids=list(range(8)), trace=trace)
    except ModuleNotFoundError:
        # axon NTFF profiling hook unavailable -- run without tracing
        res = run_bass_kernel_spmd(nc, in_maps, core_ids=list(range(8)), trace=False)
    global LAST_EXEC_NS
    LAST_EXEC_NS = res.exec_time_ns
    if trace and res.exec_time_ns is not None:
        print(f"HW exec time: {res.exec_time_ns} ns")
        if res.instructions_and_trace is not None:
            print(f"trace: {res.instructions_and_trace[1]}")

    B = query.shape[0]
    out = np.empty((B, 2 * NB, DO), dtype=np.float32)
    for c in range(8):
        b, half = c // 2, c % 2
        out[b, half * NB:(half + 1) * NB, :] = res.results[c]["out"]
    return out



# revision 7
# speedup vs baseline: 1.2198x; 1.2198x over previous
"""Multi-head attention Trainium2 Bass kernel.

Problem: B=4, N=M=2048, DM=512, H=8, DH=64, DO=512, fp32.
Sharding: 8 cores = (batch b, row-half) -- each core computes full attention
for 1024 query rows of one batch. No collectives.

Per-core dataflow (v2 -- oh flipped to [n, 65], bf16 attention operands):
  - PE-transpose Q,K,V 128x128 blocks (bf16 identity -> 1 cyc/row);
    transposed K/Q staging persists so per-head projections can be
    interleaved into later attention windows.
  - kTf/qTf [hdh, m|n] bf16 (bias + 1/sqrt(dh) folded host-side)
  - vha [m, h, 65] bf16 = [Vh + vb | 1]  (v-bias exact since sum(attn)=1)
  - scoresT[m, n] = kh @ qhT per head pair (tile_position row packing)
  - exp on ScalarE (PSUM fp32 -> SBUF bf16)
  - oh[n, 65] = ex^T(stationary) @ vha(moving, F=65); col 64 = denominator
  - normalize on DVE: per-partition reciprocal + multiply -> mh2 bf16
  - PE-transpose mh2 -> mhT [hdh, n] bf16
  - out[n, do] = sum_hp mhT_hp^T @ wp_hp + bias (ones-row matmul), PSUM->HBM
Loop nest: hp (head pair) outer, nb (n-half) inner; window w = hp*2+nb.
oh of window w-1 (+normalize+transpose) interleaves into window w's
scores/exp; V projection fills window 0; kTf/qTf head-pair projections fill
windows 1-3; output projections of nb0 fill window 7; nb1 outputs tail.
"""
import os
import sys

sys.path.insert(0, "/opt/trn_rl_repo")

import numpy as np
import ml_dtypes

import concourse.bass as bass
import concourse.mybir as mybir
import concourse.tile as tile
from concourse import bacc
from concourse.bass_utils import run_bass_kernel_spmd

F32 = mybir.dt.float32
F32R = mybir.dt.float32r
BF16 = mybir.dt.bfloat16
EXP = mybir.ActivationFunctionType.Exp
ADD = mybir.AluOpType.add
MULT = mybir.AluOpType.mult

P = 128
DM = 512
HDH = 512
DH = 64
H = 8
NB = 1024     # query rows per core
M = 2048      # kv rows
DO = 512
N_MT = M // P
N_QT = NB // P

_CACHED = {}
LAST_EXEC_NS = None


def _build():
    nc = bacc.Bacc("TRN2", target_bir_lowering=False, debug=False)

    d_q = nc.declare_dram_parameter("q", [NB, DM], F32, isOutput=False)
    d_k = nc.declare_dram_parameter("k", [M, DM], F32, isOutput=False)
    d_v = nc.declare_dram_parameter("v", [M, DM], F32, isOutput=False)
    d_wq = nc.declare_dram_parameter("wq", [DM, HDH], F32R, isOutput=False)
    d_wk = nc.declare_dram_parameter("wk", [DM, HDH], F32R, isOutput=False)
    d_wv = nc.declare_dram_parameter("wv", [DM, HDH], F32R, isOutput=False)
    d_wp = nc.declare_dram_parameter("wp", [HDH, DO], BF16, isOutput=False)
    d_qb = nc.declare_dram_parameter("qb", [P, 4], F32, isOutput=False)
    d_kb = nc.declare_dram_parameter("kb", [P, 4], F32, isOutput=False)
    d_vbrow = nc.declare_dram_parameter("vbrow", [1, HDH], F32R, isOutput=False)
    d_pb = nc.declare_dram_parameter("pb", [1, DO], F32R, isOutput=False)
    d_idb = nc.declare_dram_parameter("identb", [P, P], BF16, isOutput=False)
    d_id = nc.declare_dram_parameter("ident", [P, P], F32R, isOutput=False)
    d_ones = nc.declare_dram_parameter("ones", [P, P], F32R, isOutput=False)
    d_out = nc.declare_dram_parameter("out", [NB, DO], F32, isOutput=True)

    with tile.TileContext(nc) as tc:
        from contextlib import ExitStack
        with ExitStack() as ctx:
            persist = ctx.enter_context(tc.tile_pool(name="persist", bufs=1))
            raw = ctx.enter_context(tc.tile_pool(name="raw", bufs=5))
            vtt_pool = ctx.enter_context(tc.tile_pool(name="vtt", bufs=3))
            ex_pool = ctx.enter_context(tc.tile_pool(name="expp", bufs=20))
            nm = ctx.enter_context(tc.tile_pool(name="nm", bufs=4))
            mh2_pool = ctx.enter_context(tc.tile_pool(name="mh2", bufs=3))
            ps_sc = ctx.enter_context(tc.tile_pool(name="ps_sc", bufs=2, space="PSUM"))
            ps_oh = ctx.enter_context(tc.tile_pool(name="ps_oh", bufs=2, space="PSUM"))
            ps_pj = ctx.enter_context(tc.tile_pool(name="ps_pj", bufs=2, space="PSUM"))

            # --- constants (first DMAs out) ---
            identb = persist.tile([P, P], BF16, tag="identb", name="identb")
            nc.sync.dma_start(identb[:], d_idb[:])
            ident = persist.tile([P, P], F32R, tag="ident", name="ident")
            nc.sync.dma_start(ident[:], d_id[:])
            qb = persist.tile([P, 4], F32, tag="qb", name="qb")
            nc.sync.dma_start(qb[:], d_qb[:])
            kb = persist.tile([P, 4], F32, tag="kb", name="kb")
            nc.sync.dma_start(kb[:], d_kb[:])
            ones = persist.tile([P, P], F32R, tag="ones", name="ones")
            nc.sync.dma_start(ones[:], d_ones[:])

            # --- persistent tensors ---
            kTf = [persist.tile([P, M], BF16, tag=f"kTf{i}", name=f"kTf{i}")
                   for i in range(4)]
            qTf = [persist.tile([P, NB], BF16, tag=f"qTf{i}", name=f"qTf{i}")
                   for i in range(4)]
            ktsK = [persist.tile([P, 4, 512], F32R, tag=f"ktsK{i}", name=f"ktsK{i}")
                    for i in range(4)]
            ktsQ = [persist.tile([P, 4, 512], F32R, tag=f"ktsQ{i}", name=f"ktsQ{i}")
                    for i in range(2)]
            vha = persist.tile([P, N_MT, H, 65], BF16, tag="vha", name="vha")
            mhT = [[persist.tile([P, 512], BF16, tag=f"mhT{nb}_{hp}",
                                 name=f"mhT{nb}_{hp}")
                    for hp in range(4)] for nb in range(2)]
            vbb = persist.tile([P, H, DH], BF16, tag="vbb", name="vbb")
            pb = persist.tile([1, DO], F32R, tag="pb", name="pb")
            vbrow = persist.tile([1, HDH], F32R, tag="vbrow", name="vbrow")
            wk_sb = [persist.tile([P, HDH], F32R, tag=f"wk{d}", name=f"wk{d}")
                     for d in range(4)]
            wq_sb = [persist.tile([P, HDH], F32R, tag=f"wq{d}", name=f"wq{d}")
                     for d in range(4)]
            wv_sb = [persist.tile([P, HDH], F32R, tag=f"wv{d}", name=f"wv{d}")
                     for d in range(4)]
            wp_sb = persist.tile([P, 4, DO], BF16, tag="wp", name="wp")

            for dcc in range(4):
                nc.sync.dma_start(wk_sb[dcc][:], d_wk[dcc * P:(dcc + 1) * P, :])

            def transpose_tiles(d_src, t0, n_tiles, ts):
                """Transpose rows [t0*P, (t0+n_tiles)*P) of d_src into
                ts [P, 4, n_tiles*P] ([dm-chunk, dc, row]). Copies on ScalarE
                (idle outside the attention windows)."""
                for j in range(n_tiles):
                    rn = raw.tile([P, DM], F32R, tag="araw", name="araw")
                    nc.sync.dma_start(
                        rn[:], d_src[(t0 + j) * P:(t0 + j + 1) * P, :].bitcast(F32R))
                    pst = ps_pj.tile([P, DM], F32R, tag="pj", name="pj")
                    for dc in range(4):
                        nc.tensor.transpose(
                            pst[:, dc * P:(dc + 1) * P], rn[:, dc * P:(dc + 1) * P],
                            ident[:],
                        )
                    nc.scalar.copy(
                        ts[:, :, j * P:(j + 1) * P],
                        pst.rearrange("p (a b) -> p a b", a=4),
                    )

            def proj_k(ht, ms):
                """kTf[ht][:, ms*512:(ms+1)*512] from ktsK[ms]."""
                pp = ps_sc.tile([P, 1024], F32, tag="sc", name="sc")
                for dc in range(4):
                    nc.tensor.matmul(
                        pp[:, 0:512], wk_sb[dc][:, ht * P:(ht + 1) * P],
                        ktsK[ms][:, dc, :], start=(dc == 0), stop=(dc == 3),
                    )
                nc.vector.tensor_scalar(
                    kTf[ht][:, ms * 512:(ms + 1) * 512],
                    pp[:, 0:512], kb[:, ht:ht + 1], None, ADD,
                )

            def proj_q(ht, ns):
                pp = ps_sc.tile([P, 1024], F32, tag="sc", name="sc")
                for dc in range(4):
                    nc.tensor.matmul(
                        pp[:, 0:512], wq_sb[dc][:, ht * P:(ht + 1) * P],
                        ktsQ[ns][:, dc, :], start=(dc == 0), stop=(dc == 3),
                    )
                nc.vector.tensor_scalar(
                    qTf[ht][:, ns * 512:(ns + 1) * 512],
                    pp[:, 0:512], qb[:, ht:ht + 1], None, ADD,
                )

            # === lead-in: K transposes + kTf[0]; Q transposes + qTf[0] ===
            for ms in range(4):
                transpose_tiles(d_k, ms * 4, 4, ktsK[ms])
                proj_k(0, ms)
            for dcc in range(4):
                nc.sync.dma_start(wq_sb[dcc][:], d_wq[dcc * P:(dcc + 1) * P, :])
            for ns in range(2):
                transpose_tiles(d_q, ns * 4, 4, ktsQ[ns])
                proj_q(0, ns)
            for dcc in range(4):
                nc.sync.dma_start(wv_sb[dcc][:], d_wv[dcc * P:(dcc + 1) * P, :])
            nc.sync.dma_start(vbrow[:], d_vbrow[:])
            nc.sync.dma_start(pb[:], d_pb[:])
            for a in range(4):
                nc.sync.dma_start(wp_sb[:, a, :], d_wp[a * P:(a + 1) * P, :])
            # vbb = ones-col x vbrow: v-bias broadcast over m partitions
            bb = ps_pj.tile([P, DM], F32, tag="pj", name="pj")
            nc.tensor.matmul(bb[:], ones[0:1, 0:P], vbrow[:],
                             start=True, stop=True)
            nc.vector.tensor_copy(vbb.rearrange("p a b -> p (a b)"), bb[:])
            # ones column of vha
            nc.vector.tensor_copy(
                vha[:, :, :, 64:65],
                ones[:, 0:N_MT * H].bitcast(F32).rearrange(
                    "p (a h) -> p a h", a=N_MT)[:, :, :, None],
            )

            def emit_v_tile(mt):
                """Transpose + project one V m-tile into vha (+v-bias)."""
                vn = raw.tile([P, DM], F32R, tag="araw", name="araw")
                nc.sync.dma_start(
                    vn[:], d_v[mt * P:(mt + 1) * P, :].bitcast(F32R))
                pst = ps_pj.tile([P, DM], F32R, tag="pj", name="pj")
                for dc in range(4):
                    nc.tensor.transpose(
                        pst[:, dc * P:(dc + 1) * P], vn[:, dc * P:(dc + 1) * P],
                        ident[:],
                    )
                vtt = vtt_pool.tile([P, 4, P], F32R, tag="vtt", name="vtt")
                nc.vector.tensor_copy(vtt[:], pst.rearrange("p (a b) -> p a b", a=4))
                pp = ps_pj.tile([P, DM], F32, tag="pj", name="pj")
                for dc in range(4):
                    nc.tensor.matmul(
                        pp[:], vtt[:, dc, :], wv_sb[dc][:],
                        start=(dc == 0), stop=(dc == 3),
                    )
                nc.vector.tensor_tensor(
                    vha[:, mt, :, 0:64],
                    pp.rearrange("p (h c) -> p h c", h=H), vbb[:], ADD,
                )

            # recip consts
            from concourse.dve_ops import (
                RECIP_APPROX_FAST_CONSTS, RECIPROCAL_APPROX_FAST)
            _rc = RECIP_APPROX_FAST_CONSTS
            _mh2 = {}

            def oh_group(w, g, ex_tiles):
                """One oh accumulation group of window w: g = ab*4 + j.
                Accumulates oh[n-block j, 65] over all 16 m-tiles, then
                normalizes into mh2; emits the mh transpose after ab==1."""
                hp, nb = w // 2, w % 2
                ab, j = g // 4, g % 4
                h = 2 * hp + ab
                oh = ps_oh.tile([P, 512], F32, tag="oh", name="oh")
                for mu in range(8):
                    for jj in range(2):
                        mt = 2 * mu + jj
                        nc.tensor.matmul(
                            oh[:, 0:65],
                            ex_tiles[mu][ab][:, jj, j * P:(j + 1) * P],
                            vha[:, mt, h, :],
                            start=(mu == 0 and jj == 0),
                            stop=(mu == 7 and jj == 1),
                        )
                rr = nm.tile([P, 1], F32, tag="rr", name="rr")
                nc.vector._custom_dve(
                    RECIPROCAL_APPROX_FAST, out=rr[:], in0=oh[:, 64:65],
                    s0=_rc["s0"], s1=_rc["s1"], imm2=_rc["imm2"],
                )
                if ab == 0:
                    _mh2[j] = mh2_pool.tile([P, 2, DH], BF16, tag=f"mh2_{j}",
                                            name=f"mh2_{j}")
                mh2 = _mh2[j]
                nc.vector.tensor_scalar(
                    mh2[:, ab, :], oh[:, 0:64], rr[:, 0:1], None, MULT,
                )
                if ab == 1:
                    mtp = ps_oh.tile([P, 512], F32, tag="oh",
                                     name="oh").bitcast(BF16)[:, 0:P]
                    nc.tensor.transpose(
                        mtp, mh2.rearrange("p a b -> p (a b)"), identb[:])
                    nc.vector.tensor_copy(
                        mhT[nb][hp][:, j * P:(j + 1) * P], mtp)

            def emit_out_group(nt):
                """Output projection for global n-tile nt, PSUM -> HBM."""
                nb, jl = nt // 4, nt % 4
                po = ps_pj.tile([P, DO], F32, tag="pj", name="pj")
                for hp in range(4):
                    nc.tensor.matmul(
                        po[:], mhT[nb][hp][:, jl * P:(jl + 1) * P],
                        wp_sb[:, hp, :],
                        start=(hp == 0), stop=False, skip_group_check=True,
                    )
                nc.tensor.matmul(
                    po[:], ones[0:1, 0:P], pb[:],
                    start=False, stop=True, skip_group_check=True,
                )
                ot = nm.tile([P, DO], F32, tag="ot", name="ot")
                nc.vector.tensor_copy(ot[:], po[:])
                nc.sync.dma_start(d_out[nt * P:(nt + 1) * P, :], ot[:])

            # === attention windows ===
            prev_ex = None
            for hp in range(4):
                for nb in range(2):
                    w = hp * 2 + nb
                    ex_tiles = [[None, None] for _ in range(8)]
                    for mu in range(8):
                        for ab in range(2):
                            base = ab * 64
                            sc = ps_sc.tile([P, 1024], F32, tag="sc", name="sc")
                            for jj in range(2):
                                mt = 2 * mu + jj
                                nc.tensor.matmul(
                                    sc[:, jj * 512:(jj + 1) * 512],
                                    kTf[hp][base:base + 64, mt * P:(mt + 1) * P],
                                    qTf[hp][base:base + 64,
                                            nb * 512:(nb + 1) * 512],
                                    start=True, stop=True,
                                    tile_position=(base, 0),
                                )
                            ex = ex_pool.tile([P, 2, 512], BF16, tag="ex",
                                              name="ex")
                            nc.scalar.activation(
                                ex.rearrange("p a b -> p (a b)"), sc[:], EXP)
                            ex_tiles[mu][ab] = ex
                        # interleaved PE filler work
                        if w == 0:
                            emit_v_tile(2 * mu)
                            emit_v_tile(2 * mu + 1)
                        elif w in (1, 2, 3):
                            ht = w
                            if mu % 2 == 0:
                                proj_k(ht, mu // 2)
                            elif mu % 4 == 1:
                                proj_q(ht, mu // 4)
                        # oh of the previous window
                        if w in (1, 2, 3, 4, 5, 6):
                            oh_group(w - 1, mu, prev_ex)
                        elif w == 7:
                            if mu < 4:
                                oh_group(6, 2 * mu, prev_ex)
                                oh_group(6, 2 * mu + 1, prev_ex)
                            else:
                                emit_out_group(mu - 4)
                    prev_ex = ex_tiles

            # === tail: window 7 oh + nb1 output projections ===
            for g in range(8):
                oh_group(7, g, prev_ex)
            for nt in range(4, 8):
                emit_out_group(nt)

    nc.compile()
    return nc


def kernel(query, key, value, query_kernel, key_kernel, value_kernel,
           projection_kernel, q_bias, k_bias, v_bias, projection_bias):
    query = np.ascontiguousarray(np.asarray(query, dtype=np.float32))
    key = np.ascontiguousarray(np.asarray(key, dtype=np.float32))
    value = np.ascontiguousarray(np.asarray(value, dtype=np.float32))
    scale = np.float32(1.0 / 8.0)  # 1/sqrt(DH)

    wq = np.ascontiguousarray(
        (np.asarray(query_kernel, np.float32) * scale).transpose(1, 0, 2).reshape(DM, HDH))
    wk = np.ascontiguousarray(
        np.asarray(key_kernel, np.float32).transpose(1, 0, 2).reshape(DM, HDH))
    wv = np.ascontiguousarray(
        np.asarray(value_kernel, np.float32).transpose(1, 0, 2).reshape(DM, HDH))
    wp = np.ascontiguousarray(
        np.asarray(projection_kernel, np.float32).reshape(HDH, DO)
    ).astype(ml_dtypes.bfloat16)
    qb = np.ascontiguousarray(
        (np.asarray(q_bias, np.float32) * scale).reshape(HDH).reshape(4, P).T)
    kb = np.ascontiguousarray(np.asarray(k_bias, np.float32).reshape(HDH).reshape(4, P).T)
    vbrow = np.ascontiguousarray(np.asarray(v_bias, np.float32).reshape(1, HDH))
    pb = np.ascontiguousarray(np.asarray(projection_bias, np.float32).reshape(1, DO))
    identb = np.eye(P, dtype=ml_dtypes.bfloat16)
    ident = np.eye(P, dtype=np.float32)
    ones = np.ones((P, P), dtype=np.float32)

    if "nc" not in _CACHED:
        _CACHED["nc"] = _build()
    nc = _CACHED["nc"]

    shared = dict(wq=wq, wk=wk, wv=wv, wp=wp, qb=qb, kb=kb, vbrow=vbrow, pb=pb,
                  identb=identb, ident=ident, ones=ones)
    in_maps = []
    for c in range(8):
        b, half = c // 2, c % 2
        in_maps.append(dict(
            q=np.ascontiguousarray(query[b, half * NB:(half + 1) * NB, :]),
            k=key[b], v=value[b], **shared))

    trace = os.environ.get("KERNEL_TRACE", "0") == "1"
    try:
        res = run_bass_kernel_spmd(nc, in_maps, core_ids=list(range(8)), trace=trace)
    except ModuleNotFoundError:
        res = run_bass_kernel_spmd(nc, in_maps, core_ids=list(range(8)), trace=False)
    global LAST_EXEC_NS
    LAST_EXEC_NS = res.exec_time_ns
    if trace and res.exec_time_ns is not None:
        print(f"HW exec time: {res.exec_time_ns} ns")
        if res.instructions_and_trace is not None:
            print(f"trace: {res.instructions_and_trace[1]}")

    B = query.shape[0]
    out = np.empty((B, 2 * NB, DO), dtype=np.float32)
    for c in range(8):
        b, half = c // 2, c % 2
        out[b, half * NB:(half + 1) * NB, :] = res.results[c]["out"]
    return out


# revision 8
# speedup vs baseline: 1.2657x; 1.0376x over previous
"""Multi-head attention Trainium2 Bass kernel.

Problem: B=4, N=M=2048, DM=512, H=8, DH=64, DO=512, fp32.
Sharding: 8 cores = (batch b, row-half) -- each core computes full attention
for 1024 query rows of one batch. No collectives.

Per-core dataflow (v2 -- oh flipped to [n, 65], bf16 attention operands):
  - PE-transpose Q,K,V 128x128 blocks (bf16 identity -> 1 cyc/row);
    transposed K/Q staging persists so per-head projections can be
    interleaved into later attention windows.
  - kTf/qTf [hdh, m|n] bf16 (bias + 1/sqrt(dh) folded host-side)
  - vha [m, h, 65] bf16 = [Vh + vb | 1]  (v-bias exact since sum(attn)=1)
  - scoresT[m, n] = kh @ qhT per head pair (tile_position row packing)
  - exp on ScalarE (PSUM fp32 -> SBUF bf16)
  - oh[n, 65] = ex^T(stationary) @ vha(moving, F=65); col 64 = denominator
  - normalize on DVE: per-partition reciprocal + multiply -> mh2 bf16
  - PE-transpose mh2 -> mhT [hdh, n] bf16
  - out[n, do] = sum_hp mhT_hp^T @ wp_hp + bias (ones-row matmul), PSUM->HBM
Loop nest: hp (head pair) outer, nb (n-half) inner; window w = hp*2+nb.
oh of window w-1 (+normalize+transpose) interleaves into window w's
scores/exp; V projection fills window 0; kTf/qTf head-pair projections fill
windows 1-3; output projections of nb0 fill window 7; nb1 outputs tail.
"""
import os
import sys

sys.path.insert(0, "/opt/trn_rl_repo")

import numpy as np
import ml_dtypes

import concourse.bass as bass
import concourse.mybir as mybir
import concourse.tile as tile
from concourse import bacc
from concourse.bass_utils import run_bass_kernel_spmd

F32 = mybir.dt.float32
F32R = mybir.dt.float32r
BF16 = mybir.dt.bfloat16
EXP = mybir.ActivationFunctionType.Exp
ADD = mybir.AluOpType.add
MULT = mybir.AluOpType.mult

P = 128
DM = 512
HDH = 512
DH = 64
H = 8
NB = 1024     # query rows per core
M = 2048      # kv rows
DO = 512
N_MT = M // P
N_QT = NB // P

_CACHED = {}
LAST_EXEC_NS = None


def _build():
    nc = bacc.Bacc("TRN2", target_bir_lowering=False, debug=False)

    d_q = nc.declare_dram_parameter("q", [NB, DM], F32, isOutput=False)
    d_k = nc.declare_dram_parameter("k", [M, DM], F32, isOutput=False)
    d_v = nc.declare_dram_parameter("v", [M, DM], F32, isOutput=False)
    d_wq = nc.declare_dram_parameter("wq", [DM, HDH], F32R, isOutput=False)
    d_wk = nc.declare_dram_parameter("wk", [DM, HDH], F32R, isOutput=False)
    d_wv = nc.declare_dram_parameter("wv", [DM, HDH], F32R, isOutput=False)
    d_wp = nc.declare_dram_parameter("wp", [HDH, DO], BF16, isOutput=False)
    d_qb = nc.declare_dram_parameter("qb", [P, 4], F32, isOutput=False)
    d_kb = nc.declare_dram_parameter("kb", [P, 4], F32, isOutput=False)
    d_vbrow = nc.declare_dram_parameter("vbrow", [1, HDH], F32R, isOutput=False)
    d_pb = nc.declare_dram_parameter("pb", [1, DO], F32R, isOutput=False)
    d_idb = nc.declare_dram_parameter("identb", [P, P], BF16, isOutput=False)
    d_id = nc.declare_dram_parameter("ident", [P, P], F32R, isOutput=False)
    d_ones = nc.declare_dram_parameter("ones", [P, P], F32R, isOutput=False)
    d_out = nc.declare_dram_parameter("out", [NB, DO], F32, isOutput=True)

    with tile.TileContext(nc) as tc:
        from contextlib import ExitStack
        with ExitStack() as ctx:
            persist = ctx.enter_context(tc.tile_pool(name="persist", bufs=1))
            raw = ctx.enter_context(tc.tile_pool(name="raw", bufs=5))
            vtt_pool = ctx.enter_context(tc.tile_pool(name="vtt", bufs=3))
            ex_pool = ctx.enter_context(tc.tile_pool(name="expp", bufs=20))
            nm = ctx.enter_context(tc.tile_pool(name="nm", bufs=4))
            mh2_pool = ctx.enter_context(tc.tile_pool(name="mh2", bufs=3))
            ps_sc = ctx.enter_context(tc.tile_pool(name="ps_sc", bufs=3, space="PSUM"))
            ps_wk = ctx.enter_context(tc.tile_pool(name="ps_wk", bufs=2, space="PSUM"))

            # --- constants (first DMAs out) ---
            identb = persist.tile([P, P], BF16, tag="identb", name="identb")
            nc.sync.dma_start(identb[:], d_idb[:])
            ident = persist.tile([P, P], F32R, tag="ident", name="ident")
            nc.sync.dma_start(ident[:], d_id[:])
            qb = persist.tile([P, 4], F32, tag="qb", name="qb")
            nc.sync.dma_start(qb[:], d_qb[:])
            kb = persist.tile([P, 4], F32, tag="kb", name="kb")
            nc.sync.dma_start(kb[:], d_kb[:])
            ones = persist.tile([P, P], F32R, tag="ones", name="ones")
            nc.sync.dma_start(ones[:], d_ones[:])

            # --- persistent tensors ---
            kTf = [persist.tile([P, M], BF16, tag=f"kTf{i}", name=f"kTf{i}")
                   for i in range(4)]
            qTf = [persist.tile([P, NB], BF16, tag=f"qTf{i}", name=f"qTf{i}")
                   for i in range(4)]
            ktsK = [persist.tile([P, 4, 512], F32R, tag=f"ktsK{i}", name=f"ktsK{i}")
                    for i in range(4)]
            ktsQ = [persist.tile([P, 4, 512], F32R, tag=f"ktsQ{i}", name=f"ktsQ{i}")
                    for i in range(2)]
            vha = persist.tile([P, N_MT, H, 65], BF16, tag="vha", name="vha")
            mhT = [[persist.tile([P, 512], BF16, tag=f"mhT{nb}_{hp}",
                                 name=f"mhT{nb}_{hp}")
                    for hp in range(4)] for nb in range(2)]
            vbb = persist.tile([P, H, DH], BF16, tag="vbb", name="vbb")
            pb = persist.tile([1, DO], F32R, tag="pb", name="pb")
            vbrow = persist.tile([1, HDH], F32R, tag="vbrow", name="vbrow")
            wk_sb = [persist.tile([P, HDH], F32R, tag=f"wk{d}", name=f"wk{d}")
                     for d in range(4)]
            wq_sb = [persist.tile([P, HDH], F32R, tag=f"wq{d}", name=f"wq{d}")
                     for d in range(4)]
            wv_sb = [persist.tile([P, HDH], F32R, tag=f"wv{d}", name=f"wv{d}")
                     for d in range(4)]
            wp_sb = persist.tile([P, 4, DO], BF16, tag="wp", name="wp")

            def transpose_tiles(d_src, t0, n_tiles, ts, preloaded=None):
                """Transpose rows [t0*P, (t0+n_tiles)*P) of d_src into
                ts [P, 4, n_tiles*P] ([dm-chunk, dc, row]). Copies on ScalarE
                (idle outside the attention windows)."""
                for j in range(n_tiles):
                    if preloaded is not None:
                        rn = preloaded[j]
                    else:
                        rn = raw.tile([P, DM], F32R, tag="araw", name="araw")
                        nc.sync.dma_start(
                            rn[:], d_src[(t0 + j) * P:(t0 + j + 1) * P, :].bitcast(F32R))
                    pst = ps_wk.tile([P, DM], F32R, tag="pj", name="pj")
                    for dc in range(4):
                        nc.tensor.transpose(
                            pst[:, dc * P:(dc + 1) * P], rn[:, dc * P:(dc + 1) * P],
                            ident[:],
                        )
                    nc.scalar.copy(
                        ts[:, :, j * P:(j + 1) * P],
                        pst.rearrange("p (a b) -> p a b", a=4),
                    )

            def proj_k(ht, ms):
                """kTf[ht][:, ms*512:(ms+1)*512] from ktsK[ms]."""
                pp = ps_sc.tile([P, 1024], F32, tag="sc", name="sc")
                for dc in range(4):
                    nc.tensor.matmul(
                        pp[:, 0:512], wk_sb[dc][:, ht * P:(ht + 1) * P],
                        ktsK[ms][:, dc, :], start=(dc == 0), stop=(dc == 3),
                    )
                nc.vector.tensor_scalar(
                    kTf[ht][:, ms * 512:(ms + 1) * 512],
                    pp[:, 0:512], kb[:, ht:ht + 1], None, ADD,
                )

            def proj_q(ht, ns):
                pp = ps_sc.tile([P, 1024], F32, tag="sc", name="sc")
                for dc in range(4):
                    nc.tensor.matmul(
                        pp[:, 0:512], wq_sb[dc][:, ht * P:(ht + 1) * P],
                        ktsQ[ns][:, dc, :], start=(dc == 0), stop=(dc == 3),
                    )
                nc.vector.tensor_scalar(
                    qTf[ht][:, ns * 512:(ns + 1) * 512],
                    pp[:, 0:512], qb[:, ht:ht + 1], None, ADD,
                )

            # === lead-in: K transposes + kTf[0]; Q transposes + qTf[0] ===
            rn_k0 = []
            for j in range(4):
                rn = raw.tile([P, DM], F32R, tag="araw", name="araw")
                nc.sync.dma_start(rn[:], d_k[j * P:(j + 1) * P, :].bitcast(F32R))
                rn_k0.append(rn)
            for dcc in range(4):
                nc.sync.dma_start(wk_sb[dcc][:], d_wk[dcc * P:(dcc + 1) * P, :])
            for ms in range(4):
                transpose_tiles(d_k, ms * 4, 4, ktsK[ms],
                                preloaded=rn_k0 if ms == 0 else None)
                proj_k(0, ms)
            for dcc in range(4):
                nc.sync.dma_start(wq_sb[dcc][:], d_wq[dcc * P:(dcc + 1) * P, :])
            for ns in range(2):
                transpose_tiles(d_q, ns * 4, 4, ktsQ[ns])
                proj_q(0, ns)
            for dcc in range(4):
                nc.sync.dma_start(wv_sb[dcc][:], d_wv[dcc * P:(dcc + 1) * P, :])
            nc.sync.dma_start(vbrow[:], d_vbrow[:])
            nc.sync.dma_start(pb[:], d_pb[:])
            for a in range(4):
                nc.sync.dma_start(wp_sb[:, a, :], d_wp[a * P:(a + 1) * P, :])
            # vbb = ones-col x vbrow: v-bias broadcast over m partitions
            bb = ps_wk.tile([P, DM], F32, tag="pj", name="pj")
            nc.tensor.matmul(bb[:], ones[0:1, 0:P], vbrow[:],
                             start=True, stop=True)
            nc.vector.tensor_copy(vbb.rearrange("p a b -> p (a b)"), bb[:])
            # ones column of vha
            nc.vector.tensor_copy(
                vha[:, :, :, 64:65],
                ones[:, 0:N_MT * H].bitcast(F32).rearrange(
                    "p (a h) -> p a h", a=N_MT)[:, :, :, None],
            )

            def emit_v_tile(mt):
                """Transpose + project one V m-tile into vha (+v-bias)."""
                vn = raw.tile([P, DM], F32R, tag="araw", name="araw")
                nc.sync.dma_start(
                    vn[:], d_v[mt * P:(mt + 1) * P, :].bitcast(F32R))
                pst = ps_wk.tile([P, DM], F32R, tag="pj", name="pj")
                for dc in range(4):
                    nc.tensor.transpose(
                        pst[:, dc * P:(dc + 1) * P], vn[:, dc * P:(dc + 1) * P],
                        ident[:],
                    )
                vtt = vtt_pool.tile([P, 4, P], F32R, tag="vtt", name="vtt")
                nc.scalar.copy(vtt[:], pst.rearrange("p (a b) -> p a b", a=4))
                pp = ps_wk.tile([P, DM], F32, tag="pj", name="pj")
                for dc in range(4):
                    nc.tensor.matmul(
                        pp[:], vtt[:, dc, :], wv_sb[dc][:],
                        start=(dc == 0), stop=(dc == 3),
                    )
                nc.vector.tensor_tensor(
                    vha[:, mt, :, 0:64],
                    pp.rearrange("p (h c) -> p h c", h=H), vbb[:], ADD,
                )

            # recip consts
            from concourse.dve_ops import (
                RECIP_APPROX_FAST_CONSTS, RECIPROCAL_APPROX_FAST)
            _rc = RECIP_APPROX_FAST_CONSTS
            _mh2 = {}

            def oh_group(w, g, ex_tiles):
                """One oh accumulation group of window w: g = ab*4 + j.
                Accumulates oh[n-block j, 65] over all 16 m-tiles, then
                normalizes into mh2; emits the mh transpose after ab==1."""
                hp, nb = w // 2, w % 2
                ab, j = g // 4, g % 4
                h = 2 * hp + ab
                oh = ps_wk.tile([P, 512], F32, tag="pj", name="pj")
                for mu in range(8):
                    for jj in range(2):
                        mt = 2 * mu + jj
                        nc.tensor.matmul(
                            oh[:, 0:65],
                            ex_tiles[mu][ab][:, jj, j * P:(j + 1) * P],
                            vha[:, mt, h, :],
                            start=(mu == 0 and jj == 0),
                            stop=(mu == 7 and jj == 1),
                        )
                rr = nm.tile([P, 1], F32, tag="rr", name="rr")
                nc.vector._custom_dve(
                    RECIPROCAL_APPROX_FAST, out=rr[:], in0=oh[:, 64:65],
                    s0=_rc["s0"], s1=_rc["s1"], imm2=_rc["imm2"],
                )
                if ab == 0:
                    _mh2[j] = mh2_pool.tile([P, 2, DH], BF16, tag=f"mh2_{j}",
                                            name=f"mh2_{j}")
                mh2 = _mh2[j]
                nc.vector.tensor_scalar(
                    mh2[:, ab, :], oh[:, 0:64], rr[:, 0:1], None, MULT,
                )
                if ab == 1:
                    mtp = ps_wk.tile([P, 512], F32, tag="pj",
                                     name="pj").bitcast(BF16)[:, 0:P]
                    nc.tensor.transpose(
                        mtp, mh2.rearrange("p a b -> p (a b)"), identb[:])
                    nc.vector.tensor_copy(
                        mhT[nb][hp][:, j * P:(j + 1) * P], mtp)

            def emit_out_group(nt):
                """Output projection for global n-tile nt, PSUM -> HBM."""
                nb, jl = nt // 4, nt % 4
                po = ps_wk.tile([P, DO], F32, tag="pj", name="pj")
                for hp in range(4):
                    nc.tensor.matmul(
                        po[:], mhT[nb][hp][:, jl * P:(jl + 1) * P],
                        wp_sb[:, hp, :],
                        start=(hp == 0), stop=False, skip_group_check=True,
                    )
                nc.tensor.matmul(
                    po[:], ones[0:1, 0:P], pb[:],
                    start=False, stop=True, skip_group_check=True,
                )
                ot = nm.tile([P, DO], F32, tag="ot", name="ot")
                nc.vector.tensor_copy(ot[:], po[:])
                nc.sync.dma_start(d_out[nt * P:(nt + 1) * P, :], ot[:])

            # === attention windows ===
            prev_ex = None
            for hp in range(4):
                for nb in range(2):
                    w = hp * 2 + nb
                    ex_tiles = [[None, None] for _ in range(8)]
                    for mu in range(8):
                        for ab in range(2):
                            base = ab * 64
                            sc = ps_sc.tile([P, 1024], F32, tag="sc", name="sc")
                            for jj in range(2):
                                mt = 2 * mu + jj
                                nc.tensor.matmul(
                                    sc[:, jj * 512:(jj + 1) * 512],
                                    kTf[hp][base:base + 64, mt * P:(mt + 1) * P],
                                    qTf[hp][base:base + 64,
                                            nb * 512:(nb + 1) * 512],
                                    start=True, stop=True,
                                    tile_position=(base, 0),
                                )
                            ex = ex_pool.tile([P, 2, 512], BF16, tag="ex",
                                              name="ex")
                            nc.scalar.activation(
                                ex.rearrange("p a b -> p (a b)"), sc[:], EXP)
                            ex_tiles[mu][ab] = ex
                        # interleaved PE filler work
                        if w == 0:
                            emit_v_tile(2 * mu)
                            emit_v_tile(2 * mu + 1)
                        elif w in (1, 2, 3):
                            ht = w
                            if mu % 2 == 0:
                                proj_k(ht, mu // 2)
                            elif mu % 4 == 1:
                                proj_q(ht, mu // 4)
                        # oh of the previous window
                        if w in (1, 2, 3, 4, 5, 6):
                            oh_group(w - 1, mu, prev_ex)
                        elif w == 7:
                            if mu < 4:
                                oh_group(6, 2 * mu, prev_ex)
                                oh_group(6, 2 * mu + 1, prev_ex)
                            else:
                                emit_out_group(mu - 4)
                    prev_ex = ex_tiles

            # === tail: window 7 oh + nb1 output projections ===
            for g in range(8):
                oh_group(7, g, prev_ex)
            for nt in range(4, 8):
                emit_out_group(nt)

    nc.compile()
    return nc


def kernel(query, key, value, query_kernel, key_kernel, value_kernel,
           projection_kernel, q_bias, k_bias, v_bias, projection_bias):
    query = np.ascontiguousarray(np.asarray(query, dtype=np.float32))
    key = np.ascontiguousarray(np.asarray(key, dtype=np.float32))
    value = np.ascontiguousarray(np.asarray(value, dtype=np.float32))
    scale = np.float32(1.0 / 8.0)  # 1/sqrt(DH)

    wq = np.ascontiguousarray(
        (np.asarray(query_kernel, np.float32) * scale).transpose(1, 0, 2).reshape(DM, HDH))
    wk = np.ascontiguousarray(
        np.asarray(key_kernel, np.float32).transpose(1, 0, 2).reshape(DM, HDH))
    wv = np.ascontiguousarray(
        np.asarray(value_kernel, np.float32).transpose(1, 0, 2).reshape(DM, HDH))
    wp = np.ascontiguousarray(
        np.asarray(projection_kernel, np.float32).reshape(HDH, DO)
    ).astype(ml_dtypes.bfloat16)
    qb = np.ascontiguousarray(
        (np.asarray(q_bias, np.float32) * scale).reshape(HDH).reshape(4, P).T)
    kb = np.ascontiguousarray(np.asarray(k_bias, np.float32).reshape(HDH).reshape(4, P).T)
    vbrow = np.ascontiguousarray(np.asarray(v_bias, np.float32).reshape(1, HDH))
    pb = np.ascontiguousarray(np.asarray(projection_bias, np.float32).reshape(1, DO))
    identb = np.eye(P, dtype=ml_dtypes.bfloat16)
    ident = np.eye(P, dtype=np.float32)
    ones = np.ones((P, P), dtype=np.float32)

    if "nc" not in _CACHED:
        _CACHED["nc"] = _build()
    nc = _CACHED["nc"]

    shared = dict(wq=wq, wk=wk, wv=wv, wp=wp, qb=qb, kb=kb, vbrow=vbrow, pb=pb,
                  identb=identb, ident=ident, ones=ones)
    in_maps = []
    for c in range(8):
        b, half = c // 2, c % 2
        in_maps.append(dict(
            q=np.ascontiguousarray(query[b, half * NB:(half + 1) * NB, :]),
            k=key[b], v=value[b], **shared))

    trace = os.environ.get("KERNEL_TRACE", "0") == "1"
    try:
        res = run_bass_kernel_spmd(nc, in_maps, core_ids=list(range(8)), trace=trace)
    except ModuleNotFoundError:
        res = run_bass_kernel_spmd(nc, in_maps, core_ids=list(range(8)), trace=False)
    global LAST_EXEC_NS
    LAST_EXEC_NS = res.exec_time_ns
    if trace and res.exec_time_ns is not None:
        print(f"HW exec time: {res.exec_time_ns} ns")
        if res.instructions_and_trace is not None:
            print(f"trace: {res.instructions_and_trace[1]}")

    B = query.shape[0]
    out = np.empty((B, 2 * NB, DO), dtype=np.float32)
    for c in range(8):
        b, half = c // 2, c % 2
        out[b, half * NB:(half + 1) * NB, :] = res.results[c]["out"]
    return out


# revision 9
# speedup vs baseline: 1.2728x; 1.0056x over previous
"""Multi-head attention Trainium2 Bass kernel.

Problem: B=4, N=M=2048, DM=512, H=8, DH=64, DO=512, fp32.
Sharding: 8 cores = (batch b, row-half) -- each core computes full attention
for 1024 query rows of one batch. No collectives.

Per-core dataflow (v2 -- oh flipped to [n, 65], bf16 attention operands):
  - PE-transpose Q,K,V 128x128 blocks (bf16 identity -> 1 cyc/row);
    transposed K/Q staging persists so per-head projections can be
    interleaved into later attention windows.
  - kTf/qTf [hdh, m|n] bf16 (bias + 1/sqrt(dh) folded host-side)
  - vha [m, h, 65] bf16 = [Vh + vb | 1]  (v-bias exact since sum(attn)=1)
  - scoresT[m, n] = kh @ qhT per head pair (tile_position row packing)
  - exp on ScalarE (PSUM fp32 -> SBUF bf16)
  - oh[n, 65] = ex^T(stationary) @ vha(moving, F=65); col 64 = denominator
  - normalize on DVE: per-partition reciprocal + multiply -> mh2 bf16
  - PE-transpose mh2 -> mhT [hdh, n] bf16
  - out[n, do] = sum_hp mhT_hp^T @ wp_hp + bias (ones-row matmul), PSUM->HBM
Loop nest: hp (head pair) outer, nb (n-half) inner; window w = hp*2+nb.
oh of window w-1 (+normalize+transpose) interleaves into window w's
scores/exp; V projection fills window 0; kTf/qTf head-pair projections fill
windows 1-3; output projections of nb0 fill window 7; nb1 outputs tail.
"""
import os
import sys

sys.path.insert(0, "/opt/trn_rl_repo")

import numpy as np
import ml_dtypes

import concourse.bass as bass
import concourse.mybir as mybir
import concourse.tile as tile
from concourse import bacc
from concourse.bass_utils import run_bass_kernel_spmd

F32 = mybir.dt.float32
F32R = mybir.dt.float32r
BF16 = mybir.dt.bfloat16
EXP = mybir.ActivationFunctionType.Exp
ADD = mybir.AluOpType.add
MULT = mybir.AluOpType.mult

P = 128
DM = 512
HDH = 512
DH = 64
H = 8
NB = 1024     # query rows per core
M = 2048      # kv rows
DO = 512
N_MT = M // P
N_QT = NB // P

_CACHED = {}
LAST_EXEC_NS = None


def _build():
    nc = bacc.Bacc("TRN2", target_bir_lowering=False, debug=False)

    d_q = nc.declare_dram_parameter("q", [NB, DM], F32, isOutput=False)
    d_k = nc.declare_dram_parameter("k", [M, DM], F32, isOutput=False)
    d_v = nc.declare_dram_parameter("v", [M, DM], F32, isOutput=False)
    d_wq = nc.declare_dram_parameter("wq", [DM, HDH], F32R, isOutput=False)
    d_wk = nc.declare_dram_parameter("wk", [DM, HDH], F32R, isOutput=False)
    d_wv = nc.declare_dram_parameter("wv", [DM, HDH], F32R, isOutput=False)
    d_wp = nc.declare_dram_parameter("wp", [HDH, DO], BF16, isOutput=False)
    d_qb = nc.declare_dram_parameter("qb", [P, 4], F32, isOutput=False)
    d_kb = nc.declare_dram_parameter("kb", [P, 4], F32, isOutput=False)
    d_vbrow = nc.declare_dram_parameter("vbrow", [1, HDH], F32R, isOutput=False)
    d_pb = nc.declare_dram_parameter("pb", [1, DO], F32R, isOutput=False)
    d_idb = nc.declare_dram_parameter("identb", [P, P], BF16, isOutput=False)
    d_id = nc.declare_dram_parameter("ident", [P, P], F32R, isOutput=False)
    d_ones = nc.declare_dram_parameter("ones", [P, P], F32R, isOutput=False)
    d_out = nc.declare_dram_parameter("out", [NB, DO], F32, isOutput=True)

    with tile.TileContext(nc) as tc:
        from contextlib import ExitStack
        with ExitStack() as ctx:
            persist = ctx.enter_context(tc.tile_pool(name="persist", bufs=1))
            raw = ctx.enter_context(tc.tile_pool(name="raw", bufs=5))
            vtt_pool = ctx.enter_context(tc.tile_pool(name="vtt", bufs=3))
            ex_pool = ctx.enter_context(tc.tile_pool(name="expp", bufs=20))
            nm = ctx.enter_context(tc.tile_pool(name="nm", bufs=4))
            mh2_pool = ctx.enter_context(tc.tile_pool(name="mh2", bufs=3))
            ps_sc = ctx.enter_context(tc.tile_pool(name="ps_sc", bufs=3, space="PSUM"))
            ps_wk = ctx.enter_context(tc.tile_pool(name="ps_wk", bufs=2, space="PSUM"))

            # --- constants (first DMAs out) ---
            identb = persist.tile([P, P], BF16, tag="identb", name="identb")
            nc.sync.dma_start(identb[:], d_idb[:])
            ident = persist.tile([P, P], F32R, tag="ident", name="ident")
            nc.sync.dma_start(ident[:], d_id[:])
            qb = persist.tile([P, 4], F32, tag="qb", name="qb")
            nc.sync.dma_start(qb[:], d_qb[:])
            kb = persist.tile([P, 4], F32, tag="kb", name="kb")
            nc.sync.dma_start(kb[:], d_kb[:])
            ones = persist.tile([P, P], F32R, tag="ones", name="ones")
            nc.sync.dma_start(ones[:], d_ones[:])

            # --- persistent tensors ---
            kTf = [persist.tile([P, M], BF16, tag=f"kTf{i}", name=f"kTf{i}")
                   for i in range(4)]
            qTf = [persist.tile([P, NB], BF16, tag=f"qTf{i}", name=f"qTf{i}")
                   for i in range(4)]
            ktsK = [persist.tile([P, 4, 512], F32R, tag=f"ktsK{i}", name=f"ktsK{i}")
                    for i in range(4)]
            ktsQ = [persist.tile([P, 4, 512], F32R, tag=f"ktsQ{i}", name=f"ktsQ{i}")
                    for i in range(2)]
            vha = persist.tile([P, N_MT, H, 65], BF16, tag="vha", name="vha")
            mhT = [[persist.tile([P, 512], BF16, tag=f"mhT{nb}_{hp}",
                                 name=f"mhT{nb}_{hp}")
                    for hp in range(4)] for nb in range(2)]
            vbb = persist.tile([P, H, DH], BF16, tag="vbb", name="vbb")
            pb = persist.tile([1, DO], F32R, tag="pb", name="pb")
            vbrow = persist.tile([1, HDH], F32R, tag="vbrow", name="vbrow")
            wk_sb = [persist.tile([P, HDH], F32R, tag=f"wk{d}", name=f"wk{d}")
                     for d in range(4)]
            wq_sb = [persist.tile([P, HDH], F32R, tag=f"wq{d}", name=f"wq{d}")
                     for d in range(4)]
            wv_sb = [persist.tile([P, HDH], F32R, tag=f"wv{d}", name=f"wv{d}")
                     for d in range(4)]
            wp_sb = persist.tile([P, 4, DO], BF16, tag="wp", name="wp")

            def transpose_tiles(d_src, t0, n_tiles, ts, preloaded=None):
                """Transpose rows [t0*P, (t0+n_tiles)*P) of d_src into
                ts [P, 4, n_tiles*P] ([dm-chunk, dc, row]). Copies on ScalarE
                (idle outside the attention windows)."""
                for j in range(n_tiles):
                    if preloaded is not None:
                        rn = preloaded[j]
                    else:
                        rn = raw.tile([P, DM], F32R, tag="araw", name="araw")
                        nc.sync.dma_start(
                            rn[:], d_src[(t0 + j) * P:(t0 + j + 1) * P, :].bitcast(F32R))
                    pst = ps_wk.tile([P, DM], F32R, tag="pj", name="pj")
                    for dc in range(4):
                        nc.tensor.transpose(
                            pst[:, dc * P:(dc + 1) * P], rn[:, dc * P:(dc + 1) * P],
                            ident[:],
                        )
                    nc.scalar.copy(
                        ts[:, :, j * P:(j + 1) * P],
                        pst.rearrange("p (a b) -> p a b", a=4),
                    )

            def proj_k(ht, ms):
                """kTf[ht][:, ms*512:(ms+1)*512] from ktsK[ms]."""
                pp = ps_sc.tile([P, 1024], F32, tag="sc", name="sc")
                for dc in range(4):
                    nc.tensor.matmul(
                        pp[:, 0:512], wk_sb[dc][:, ht * P:(ht + 1) * P],
                        ktsK[ms][:, dc, :], start=(dc == 0), stop=(dc == 3),
                    )
                nc.vector.tensor_scalar(
                    kTf[ht][:, ms * 512:(ms + 1) * 512],
                    pp[:, 0:512], kb[:, ht:ht + 1], None, ADD,
                )

            def proj_q(ht, ns):
                pp = ps_sc.tile([P, 1024], F32, tag="sc", name="sc")
                for dc in range(4):
                    nc.tensor.matmul(
                        pp[:, 0:512], wq_sb[dc][:, ht * P:(ht + 1) * P],
                        ktsQ[ns][:, dc, :], start=(dc == 0), stop=(dc == 3),
                    )
                nc.vector.tensor_scalar(
                    qTf[ht][:, ns * 512:(ns + 1) * 512],
                    pp[:, 0:512], qb[:, ht:ht + 1], None, ADD,
                )

            # === lead-in: K transposes + kTf[0]; Q transposes + qTf[0] ===
            rn_k0 = []
            for j in range(4):
                rn = raw.tile([P, DM], F32R, tag="araw", name="araw")
                nc.sync.dma_start(rn[:], d_k[j * P:(j + 1) * P, :].bitcast(F32R))
                rn_k0.append(rn)
            for dcc in range(4):
                nc.sync.dma_start(wk_sb[dcc][:], d_wk[dcc * P:(dcc + 1) * P, :])
            for ms in range(4):
                transpose_tiles(d_k, ms * 4, 4, ktsK[ms],
                                preloaded=rn_k0 if ms == 0 else None)
                proj_k(0, ms)
            for dcc in range(4):
                nc.sync.dma_start(wq_sb[dcc][:], d_wq[dcc * P:(dcc + 1) * P, :])
            for ns in range(2):
                transpose_tiles(d_q, ns * 4, 4, ktsQ[ns])
                proj_q(0, ns)
            for dcc in range(4):
                nc.sync.dma_start(wv_sb[dcc][:], d_wv[dcc * P:(dcc + 1) * P, :])
            nc.sync.dma_start(vbrow[:], d_vbrow[:])
            nc.sync.dma_start(pb[:], d_pb[:])
            for a in range(4):
                nc.sync.dma_start(wp_sb[:, a, :], d_wp[a * P:(a + 1) * P, :])
            # vbb = ones-col x vbrow: v-bias broadcast over m partitions
            bb = ps_wk.tile([P, DM], F32, tag="pj", name="pj")
            nc.tensor.matmul(bb[:], ones[0:1, 0:P], vbrow[:],
                             start=True, stop=True)
            nc.vector.tensor_copy(vbb.rearrange("p a b -> p (a b)"), bb[:])
            # ones column of vha
            nc.vector.tensor_copy(
                vha[:, :, :, 64:65],
                ones[:, 0:N_MT * H].bitcast(F32).rearrange(
                    "p (a h) -> p a h", a=N_MT)[:, :, :, None],
            )

            def emit_v_tile(mt):
                """Transpose + project one V m-tile into vha (+v-bias)."""
                vn = raw.tile([P, DM], F32R, tag="araw", name="araw")
                nc.sync.dma_start(
                    vn[:], d_v[mt * P:(mt + 1) * P, :].bitcast(F32R))
                pst = ps_wk.tile([P, DM], F32R, tag="pj", name="pj")
                for dc in range(4):
                    nc.tensor.transpose(
                        pst[:, dc * P:(dc + 1) * P], vn[:, dc * P:(dc + 1) * P],
                        ident[:],
                    )
                vtt = vtt_pool.tile([P, 4, P], F32R, tag="vtt", name="vtt")
                nc.vector.tensor_copy(vtt[:], pst.rearrange("p (a b) -> p a b", a=4))
                pp = ps_wk.tile([P, DM], F32, tag="pj", name="pj")
                for dc in range(4):
                    nc.tensor.matmul(
                        pp[:], vtt[:, dc, :], wv_sb[dc][:],
                        start=(dc == 0), stop=(dc == 3),
                    )
                nc.vector.tensor_tensor(
                    vha[:, mt, :, 0:64],
                    pp.rearrange("p (h c) -> p h c", h=H), vbb[:], ADD,
                )

            # recip consts
            from concourse.dve_ops import (
                RECIP_APPROX_FAST_CONSTS, RECIPROCAL_APPROX_FAST)
            _rc = RECIP_APPROX_FAST_CONSTS
            _mh2 = {}

            def oh_group(w, g, ex_tiles):
                """One oh accumulation group of window w: g = ab*4 + j.
                Accumulates oh[n-block j, 65] over all 16 m-tiles, then
                normalizes into mh2; emits the mh transpose after ab==1."""
                hp, nb = w // 2, w % 2
                ab, j = g // 4, g % 4
                h = 2 * hp + ab
                oh = ps_wk.tile([P, 512], F32, tag="pj", name="pj")
                for mu in range(8):
                    for jj in range(2):
                        mt = 2 * mu + jj
                        nc.tensor.matmul(
                            oh[:, 0:65],
                            ex_tiles[mu][ab][:, jj, j * P:(j + 1) * P],
                            vha[:, mt, h, :],
                            start=(mu == 0 and jj == 0),
                            stop=(mu == 7 and jj == 1),
                        )
                rr = nm.tile([P, 1], F32, tag="rr", name="rr")
                nc.vector._custom_dve(
                    RECIPROCAL_APPROX_FAST, out=rr[:], in0=oh[:, 64:65],
                    s0=_rc["s0"], s1=_rc["s1"], imm2=_rc["imm2"],
                )
                if ab == 0:
                    _mh2[j] = mh2_pool.tile([P, 2, DH], BF16, tag=f"mh2_{j}",
                                            name=f"mh2_{j}")
                mh2 = _mh2[j]
                nc.vector.tensor_scalar(
                    mh2[:, ab, :], oh[:, 0:64], rr[:, 0:1], None, MULT,
                )
                if ab == 1:
                    mtp = ps_wk.tile([P, 512], F32, tag="pj",
                                     name="pj").bitcast(BF16)[:, 0:P]
                    nc.tensor.transpose(
                        mtp, mh2.rearrange("p a b -> p (a b)"), identb[:])
                    nc.vector.tensor_copy(
                        mhT[nb][hp][:, j * P:(j + 1) * P], mtp)

            def emit_out_group(nt):
                """Output projection for global n-tile nt, PSUM -> HBM."""
                nb, jl = nt // 4, nt % 4
                po = ps_wk.tile([P, DO], F32, tag="pj", name="pj")
                for hp in range(4):
                    nc.tensor.matmul(
                        po[:], mhT[nb][hp][:, jl * P:(jl + 1) * P],
                        wp_sb[:, hp, :],
                        start=(hp == 0), stop=False, skip_group_check=True,
                    )
                nc.tensor.matmul(
                    po[:], ones[0:1, 0:P], pb[:],
                    start=False, stop=True, skip_group_check=True,
                )
                ot = nm.tile([P, DO], F32, tag="ot", name="ot")
                nc.vector.tensor_copy(ot[:], po[:])
                nc.sync.dma_start(d_out[nt * P:(nt + 1) * P, :], ot[:])

            # === attention windows ===
            prev_ex = None
            for hp in range(4):
                for nb in range(2):
                    w = hp * 2 + nb
                    ex_tiles = [[None, None] for _ in range(8)]
                    for mu in range(8):
                        for ab in range(2):
                            base = ab * 64
                            sc = ps_sc.tile([P, 1024], F32, tag="sc", name="sc")
                            for jj in range(2):
                                mt = 2 * mu + jj
                                nc.tensor.matmul(
                                    sc[:, jj * 512:(jj + 1) * 512],
                                    kTf[hp][base:base + 64, mt * P:(mt + 1) * P],
                                    qTf[hp][base:base + 64,
                                            nb * 512:(nb + 1) * 512],
                                    start=True, stop=True,
                                    tile_position=(base, 0),
                                )
                            ex = ex_pool.tile([P, 2, 512], BF16, tag="ex",
                                              name="ex")
                            nc.scalar.activation(
                                ex.rearrange("p a b -> p (a b)"), sc[:], EXP)
                            ex_tiles[mu][ab] = ex
                        # interleaved PE filler work
                        if w == 0:
                            emit_v_tile(2 * mu)
                            emit_v_tile(2 * mu + 1)
                            if mu % 2 == 1:
                                proj_k(1, mu // 2)
                        elif w == 1:
                            if mu % 4 == 1:
                                proj_q(1, mu // 4)
                        elif w in (2, 3):
                            ht = w
                            if mu % 2 == 0:
                                proj_k(ht, mu // 2)
                            elif mu % 4 == 1:
                                proj_q(ht, mu // 4)
                        # oh of the previous window
                        if w in (1, 2, 3, 4, 5, 6):
                            oh_group(w - 1, mu, prev_ex)
                        elif w == 7:
                            if mu < 4:
                                oh_group(6, 2 * mu, prev_ex)
                                oh_group(6, 2 * mu + 1, prev_ex)
                            else:
                                emit_out_group(mu - 4)
                    prev_ex = ex_tiles

            # === tail: window 7 oh + nb1 output projections ===
            for g in range(8):
                oh_group(7, g, prev_ex)
            for nt in range(4, 8):
                emit_out_group(nt)

    nc.compile()
    return nc


def kernel(query, key, value, query_kernel, key_kernel, value_kernel,
           projection_kernel, q_bias, k_bias, v_bias, projection_bias):
    query = np.ascontiguousarray(np.asarray(query, dtype=np.float32))
    key = np.ascontiguousarray(np.asarray(key, dtype=np.float32))
    value = np.ascontiguousarray(np.asarray(value, dtype=np.float32))
    scale = np.float32(1.0 / 8.0)  # 1/sqrt(DH)

    wq = np.ascontiguousarray(
        (np.asarray(query_kernel, np.float32) * scale).transpose(1, 0, 2).reshape(DM, HDH))
    wk = np.ascontiguousarray(
        np.asarray(key_kernel, np.float32).transpose(1, 0, 2).reshape(DM, HDH))
    wv = np.ascontiguousarray(
        np.asarray(value_kernel, np.float32).transpose(1, 0, 2).reshape(DM, HDH))
    wp = np.ascontiguousarray(
        np.asarray(projection_kernel, np.float32).reshape(HDH, DO)
    ).astype(ml_dtypes.bfloat16)
    qb = np.ascontiguousarray(
        (np.asarray(q_bias, np.float32) * scale).reshape(HDH).reshape(4, P).T)
    kb = np.ascontiguousarray(np.asarray(k_bias, np.float32).reshape(HDH).reshape(4, P).T)
    vbrow = np.ascontiguousarray(np.asarray(v_bias, np.float32).reshape(1, HDH))
    pb = np.ascontiguousarray(np.asarray(projection_bias, np.float32).reshape(1, DO))
    identb = np.eye(P, dtype=ml_dtypes.bfloat16)
    ident = np.eye(P, dtype=np.float32)
    ones = np.ones((P, P), dtype=np.float32)

    if "nc" not in _CACHED:
        _CACHED["nc"] = _build()
    nc = _CACHED["nc"]

    shared = dict(wq=wq, wk=wk, wv=wv, wp=wp, qb=qb, kb=kb, vbrow=vbrow, pb=pb,
                  identb=identb, ident=ident, ones=ones)
    in_maps = []
    for c in range(8):
        b, half = c // 2, c % 2
        in_maps.append(dict(
            q=np.ascontiguousarray(query[b, half * NB:(half + 1) * NB, :]),
            k=key[b], v=value[b], **shared))

    trace = os.environ.get("KERNEL_TRACE", "0") == "1"
    try:
        res = run_bass_kernel_spmd(nc, in_maps, core_ids=list(range(8)), trace=trace)
    except ModuleNotFoundError:
        res = run_bass_kernel_spmd(nc, in_maps, core_ids=list(range(8)), trace=False)
    global LAST_EXEC_NS
    LAST_EXEC_NS = res.exec_time_ns
    if trace and res.exec_time_ns is not None:
        print(f"HW exec time: {res.exec_time_ns} ns")
        if res.instructions_and_trace is not None:
            print(f"trace: {res.instructions_and_trace[1]}")

    B = query.shape[0]
    out = np.empty((B, 2 * NB, DO), dtype=np.float32)
    for c in range(8):
        b, half = c // 2, c % 2
        out[b, half * NB:(half + 1) * NB, :] = res.results[c]["out"]
    return out


# revision 10
# speedup vs baseline: 1.2736x; 1.0007x over previous
"""Multi-head attention Trainium2 Bass kernel.

Problem: B=4, N=M=2048, DM=512, H=8, DH=64, DO=512, fp32.
Sharding: 8 cores = (batch b, row-half) -- each core computes full attention
for 1024 query rows of one batch. No collectives.

Per-core dataflow (v2 -- oh flipped to [n, 65], bf16 attention operands):
  - PE-transpose Q,K,V 128x128 blocks (bf16 identity -> 1 cyc/row);
    transposed K/Q staging persists so per-head projections can be
    interleaved into later attention windows.
  - kTf/qTf [hdh, m|n] bf16 (bias + 1/sqrt(dh) folded host-side)
  - vha [m, h, 65] bf16 = [Vh + vb | 1]  (v-bias exact since sum(attn)=1)
  - scoresT[m, n] = kh @ qhT per head pair (tile_position row packing)
  - exp on ScalarE (PSUM fp32 -> SBUF bf16)
  - oh[n, 65] = ex^T(stationary) @ vha(moving, F=65); col 64 = denominator
  - normalize on DVE: per-partition reciprocal + multiply -> mh2 bf16
  - PE-transpose mh2 -> mhT [hdh, n] bf16
  - out[n, do] = sum_hp mhT_hp^T @ wp_hp + bias (ones-row matmul), PSUM->HBM
Loop nest: hp (head pair) outer, nb (n-half) inner; window w = hp*2+nb.
oh of window w-1 (+normalize+transpose) interleaves into window w's
scores/exp; V projection fills window 0; kTf/qTf head-pair projections fill
windows 1-3; output projections of nb0 fill window 7; nb1 outputs tail.
"""
import os
import sys

sys.path.insert(0, "/opt/trn_rl_repo")

import numpy as np
import ml_dtypes

import concourse.bass as bass
import concourse.mybir as mybir
import concourse.tile as tile
from concourse import bacc
from concourse.bass_utils import run_bass_kernel_spmd

F32 = mybir.dt.float32
F32R = mybir.dt.float32r
BF16 = mybir.dt.bfloat16
EXP = mybir.ActivationFunctionType.Exp
ADD = mybir.AluOpType.add
MULT = mybir.AluOpType.mult

P = 128
DM = 512
HDH = 512
DH = 64
H = 8
NB = 1024     # query rows per core
M = 2048      # kv rows
DO = 512
N_MT = M // P
N_QT = NB // P

_CACHED = {}
LAST_EXEC_NS = None


def _build():
    nc = bacc.Bacc("TRN2", target_bir_lowering=False, debug=False)

    d_q = nc.declare_dram_parameter("q", [NB, DM], F32, isOutput=False)
    d_k = nc.declare_dram_parameter("k", [M, DM], F32, isOutput=False)
    d_v = nc.declare_dram_parameter("v", [M, DM], F32, isOutput=False)
    d_wq = nc.declare_dram_parameter("wq", [DM, HDH], F32R, isOutput=False)
    d_wk = nc.declare_dram_parameter("wk", [DM, HDH], F32R, isOutput=False)
    d_wv = nc.declare_dram_parameter("wv", [DM, HDH], F32R, isOutput=False)
    d_wp = nc.declare_dram_parameter("wp", [HDH, DO], BF16, isOutput=False)
    d_qb = nc.declare_dram_parameter("qb", [P, 4], F32, isOutput=False)
    d_kb = nc.declare_dram_parameter("kb", [P, 4], F32, isOutput=False)
    d_vbrow = nc.declare_dram_parameter("vbrow", [1, HDH], F32R, isOutput=False)
    d_pb = nc.declare_dram_parameter("pb", [1, DO], F32R, isOutput=False)
    d_idb = nc.declare_dram_parameter("identb", [P, P], BF16, isOutput=False)
    d_id = nc.declare_dram_parameter("ident", [P, P], F32R, isOutput=False)
    d_ones = nc.declare_dram_parameter("ones", [P, P], F32R, isOutput=False)
    d_out = nc.declare_dram_parameter("out", [NB, DO], F32, isOutput=True)

    with tile.TileContext(nc) as tc:
        from contextlib import ExitStack
        with ExitStack() as ctx:
            persist = ctx.enter_context(tc.tile_pool(name="persist", bufs=1))
            raw = ctx.enter_context(tc.tile_pool(name="raw", bufs=5))
            vtt_pool = ctx.enter_context(tc.tile_pool(name="vtt", bufs=3))
            ex_pool = ctx.enter_context(tc.tile_pool(name="expp", bufs=20))
            nm = ctx.enter_context(tc.tile_pool(name="nm", bufs=4))
            mh2_pool = ctx.enter_context(tc.tile_pool(name="mh2", bufs=3))
            ps_sc = ctx.enter_context(tc.tile_pool(name="ps_sc", bufs=3, space="PSUM"))
            ps_wk = ctx.enter_context(tc.tile_pool(name="ps_wk", bufs=2, space="PSUM"))

            # --- constants (first DMAs out) ---
            identb = persist.tile([P, P], BF16, tag="identb", name="identb")
            nc.sync.dma_start(identb[:], d_idb[:])
            ident = persist.tile([P, P], F32R, tag="ident", name="ident")
            nc.sync.dma_start(ident[:], d_id[:])
            qb = persist.tile([P, 4], F32, tag="qb", name="qb")
            nc.sync.dma_start(qb[:], d_qb[:])
            kb = persist.tile([P, 4], F32, tag="kb", name="kb")
            nc.sync.dma_start(kb[:], d_kb[:])
            ones = persist.tile([P, P], F32R, tag="ones", name="ones")
            nc.sync.dma_start(ones[:], d_ones[:])

            # --- persistent tensors ---
            kTf = [persist.tile([P, M], BF16, tag=f"kTf{i}", name=f"kTf{i}")
                   for i in range(4)]
            qTf = [persist.tile([P, NB], BF16, tag=f"qTf{i}", name=f"qTf{i}")
                   for i in range(4)]
            ktsK = [persist.tile([P, 4, 512], F32R, tag=f"ktsK{i}", name=f"ktsK{i}")
                    for i in range(4)]
            ktsQ = [persist.tile([P, 4, 512], F32R, tag=f"ktsQ{i}", name=f"ktsQ{i}")
                    for i in range(2)]
            vha = persist.tile([P, N_MT, H, 65], BF16, tag="vha", name="vha")
            mhT = [[persist.tile([P, 512], BF16, tag=f"mhT{nb}_{hp}",
                                 name=f"mhT{nb}_{hp}")
                    for hp in range(4)] for nb in range(2)]
            vbb = persist.tile([P, H, DH], BF16, tag="vbb", name="vbb")
            pb = persist.tile([1, DO], F32R, tag="pb", name="pb")
            vbrow = persist.tile([1, HDH], F32R, tag="vbrow", name="vbrow")
            wk_sb = [persist.tile([P, HDH], F32R, tag=f"wk{d}", name=f"wk{d}")
                     for d in range(4)]
            wq_sb = [persist.tile([P, HDH], F32R, tag=f"wq{d}", name=f"wq{d}")
                     for d in range(4)]
            wv_sb = [persist.tile([P, HDH], F32R, tag=f"wv{d}", name=f"wv{d}")
                     for d in range(4)]
            wp_sb = persist.tile([P, 4, DO], BF16, tag="wp", name="wp")

            def transpose_tiles(d_src, t0, n_tiles, ts, preloaded=None, eng=None):
                """Transpose rows [t0*P, (t0+n_tiles)*P) of d_src into
                ts [P, 4, n_tiles*P] ([dm-chunk, dc, row]). Copies on ScalarE
                (idle outside the attention windows)."""
                for j in range(n_tiles):
                    if preloaded is not None:
                        rn = preloaded[j]
                    else:
                        rn = raw.tile([P, DM], F32R, tag="araw", name="araw")
                        (eng or nc.sync).dma_start(
                            rn[:], d_src[(t0 + j) * P:(t0 + j + 1) * P, :].bitcast(F32R))
                    pst = ps_wk.tile([P, DM], F32R, tag="pj", name="pj")
                    for dc in range(4):
                        nc.tensor.transpose(
                            pst[:, dc * P:(dc + 1) * P], rn[:, dc * P:(dc + 1) * P],
                            ident[:],
                        )
                    nc.scalar.copy(
                        ts[:, :, j * P:(j + 1) * P],
                        pst.rearrange("p (a b) -> p a b", a=4),
                    )

            def proj_k(ht, ms):
                """kTf[ht][:, ms*512:(ms+1)*512] from ktsK[ms]."""
                pp = ps_sc.tile([P, 1024], F32, tag="sc", name="sc")
                for dc in range(4):
                    nc.tensor.matmul(
                        pp[:, 0:512], wk_sb[dc][:, ht * P:(ht + 1) * P],
                        ktsK[ms][:, dc, :], start=(dc == 0), stop=(dc == 3),
                    )
                nc.vector.tensor_scalar(
                    kTf[ht][:, ms * 512:(ms + 1) * 512],
                    pp[:, 0:512], kb[:, ht:ht + 1], None, ADD,
                )

            def proj_q(ht, ns):
                pp = ps_sc.tile([P, 1024], F32, tag="sc", name="sc")
                for dc in range(4):
                    nc.tensor.matmul(
                        pp[:, 0:512], wq_sb[dc][:, ht * P:(ht + 1) * P],
                        ktsQ[ns][:, dc, :], start=(dc == 0), stop=(dc == 3),
                    )
                nc.vector.tensor_scalar(
                    qTf[ht][:, ns * 512:(ns + 1) * 512],
                    pp[:, 0:512], qb[:, ht:ht + 1], None, ADD,
                )

            # === lead-in: K transposes + kTf[0]; Q transposes + qTf[0] ===
            rn_k0 = []
            for j in range(4):
                rn = raw.tile([P, DM], F32R, tag="araw", name="araw")
                nc.sync.dma_start(rn[:], d_k[j * P:(j + 1) * P, :].bitcast(F32R))
                rn_k0.append(rn)
            for dcc in range(4):
                nc.sync.dma_start(wk_sb[dcc][:], d_wk[dcc * P:(dcc + 1) * P, :])
            for ms in range(4):
                transpose_tiles(d_k, ms * 4, 4, ktsK[ms],
                                preloaded=rn_k0 if ms == 0 else None,
                                eng=nc.gpsimd if ms >= 2 else nc.sync)
                proj_k(0, ms)
            for dcc in range(4):
                nc.gpsimd.dma_start(wq_sb[dcc][:], d_wq[dcc * P:(dcc + 1) * P, :])
            for ns in range(2):
                transpose_tiles(d_q, ns * 4, 4, ktsQ[ns])
                proj_q(0, ns)
            for dcc in range(4):
                nc.gpsimd.dma_start(wv_sb[dcc][:], d_wv[dcc * P:(dcc + 1) * P, :])
            nc.gpsimd.dma_start(vbrow[:], d_vbrow[:])
            nc.gpsimd.dma_start(pb[:], d_pb[:])
            for a in range(4):
                nc.gpsimd.dma_start(wp_sb[:, a, :], d_wp[a * P:(a + 1) * P, :])
            # vbb = ones-col x vbrow: v-bias broadcast over m partitions
            bb = ps_wk.tile([P, DM], F32, tag="pj", name="pj")
            nc.tensor.matmul(bb[:], ones[0:1, 0:P], vbrow[:],
                             start=True, stop=True)
            nc.vector.tensor_copy(vbb.rearrange("p a b -> p (a b)"), bb[:])
            # ones column of vha
            nc.vector.tensor_copy(
                vha[:, :, :, 64:65],
                ones[:, 0:N_MT * H].bitcast(F32).rearrange(
                    "p (a h) -> p a h", a=N_MT)[:, :, :, None],
            )

            def emit_v_tile(mt):
                """Transpose + project one V m-tile into vha (+v-bias)."""
                vn = raw.tile([P, DM], F32R, tag="araw", name="araw")
                (nc.gpsimd if mt % 2 else nc.sync).dma_start(
                    vn[:], d_v[mt * P:(mt + 1) * P, :].bitcast(F32R))
                pst = ps_wk.tile([P, DM], F32R, tag="pj", name="pj")
                for dc in range(4):
                    nc.tensor.transpose(
                        pst[:, dc * P:(dc + 1) * P], vn[:, dc * P:(dc + 1) * P],
                        ident[:],
                    )
                vtt = vtt_pool.tile([P, 4, P], F32R, tag="vtt", name="vtt")
                nc.vector.tensor_copy(vtt[:], pst.rearrange("p (a b) -> p a b", a=4))
                pp = ps_wk.tile([P, DM], F32, tag="pj", name="pj")
                for dc in range(4):
                    nc.tensor.matmul(
                        pp[:], vtt[:, dc, :], wv_sb[dc][:],
                        start=(dc == 0), stop=(dc == 3),
                    )
                nc.vector.tensor_tensor(
                    vha[:, mt, :, 0:64],
                    pp.rearrange("p (h c) -> p h c", h=H), vbb[:], ADD,
                )

            # recip consts
            from concourse.dve_ops import (
                RECIP_APPROX_FAST_CONSTS, RECIPROCAL_APPROX_FAST)
            _rc = RECIP_APPROX_FAST_CONSTS
            _mh2 = {}

            def oh_group(w, g, ex_tiles):
                """One oh accumulation group of window w: g = ab*4 + j.
                Accumulates oh[n-block j, 65] over all 16 m-tiles, then
                normalizes into mh2; emits the mh transpose after ab==1."""
                hp, nb = w // 2, w % 2
                ab, j = g // 4, g % 4
                h = 2 * hp + ab
                oh = ps_wk.tile([P, 512], F32, tag="pj", name="pj")
                for mu in range(8):
                    for jj in range(2):
                        mt = 2 * mu + jj
                        nc.tensor.matmul(
                            oh[:, 0:65],
                            ex_tiles[mu][ab][:, jj, j * P:(j + 1) * P],
                            vha[:, mt, h, :],
                            start=(mu == 0 and jj == 0),
                            stop=(mu == 7 and jj == 1),
                        )
                rr = nm.tile([P, 1], F32, tag="rr", name="rr")
                nc.vector._custom_dve(
                    RECIPROCAL_APPROX_FAST, out=rr[:], in0=oh[:, 64:65],
                    s0=_rc["s0"], s1=_rc["s1"], imm2=_rc["imm2"],
                )
                if ab == 0:
                    _mh2[j] = mh2_pool.tile([P, 2, DH], BF16, tag=f"mh2_{j}",
                                            name=f"mh2_{j}")
                mh2 = _mh2[j]
                nc.vector.tensor_scalar(
                    mh2[:, ab, :], oh[:, 0:64], rr[:, 0:1], None, MULT,
                )
                if ab == 1:
                    mtp = ps_wk.tile([P, 512], F32, tag="pj",
                                     name="pj").bitcast(BF16)[:, 0:P]
                    nc.tensor.transpose(
                        mtp, mh2.rearrange("p a b -> p (a b)"), identb[:])
                    nc.vector.tensor_copy(
                        mhT[nb][hp][:, j * P:(j + 1) * P], mtp)

            def emit_out_group(nt):
                """Output projection for global n-tile nt, PSUM -> HBM."""
                nb, jl = nt // 4, nt % 4
                po = ps_wk.tile([P, DO], F32, tag="pj", name="pj")
                for hp in range(4):
                    nc.tensor.matmul(
                        po[:], mhT[nb][hp][:, jl * P:(jl + 1) * P],
                        wp_sb[:, hp, :],
                        start=(hp == 0), stop=False, skip_group_check=True,
                    )
                nc.tensor.matmul(
                    po[:], ones[0:1, 0:P], pb[:],
                    start=False, stop=True, skip_group_check=True,
                )
                ot = nm.tile([P, DO], F32, tag="ot", name="ot")
                nc.vector.tensor_copy(ot[:], po[:])
                nc.sync.dma_start(d_out[nt * P:(nt + 1) * P, :], ot[:])

            # === attention windows ===
            prev_ex = None
            for hp in range(4):
                for nb in range(2):
                    w = hp * 2 + nb
                    ex_tiles = [[None, None] for _ in range(8)]
                    for mu in range(8):
                        for ab in range(2):
                            base = ab * 64
                            sc = ps_sc.tile([P, 1024], F32, tag="sc", name="sc")
                            for jj in range(2):
                                mt = 2 * mu + jj
                                nc.tensor.matmul(
                                    sc[:, jj * 512:(jj + 1) * 512],
                                    kTf[hp][base:base + 64, mt * P:(mt + 1) * P],
                                    qTf[hp][base:base + 64,
                                            nb * 512:(nb + 1) * 512],
                                    start=True, stop=True,
                                    tile_position=(base, 0),
                                )
                            ex = ex_pool.tile([P, 2, 512], BF16, tag="ex",
                                              name="ex")
                            nc.scalar.activation(
                                ex.rearrange("p a b -> p (a b)"), sc[:], EXP)
                            ex_tiles[mu][ab] = ex
                        # interleaved PE filler work
                        if w == 0:
                            emit_v_tile(2 * mu)
                            emit_v_tile(2 * mu + 1)
                            if mu % 2 == 1:
                                proj_k(1, mu // 2)
                        elif w == 1:
                            if mu % 4 == 1:
                                proj_q(1, mu // 4)
                        elif w in (2, 3):
                            ht = w
                            if mu % 2 == 0:
                                proj_k(ht, mu // 2)
                            elif mu % 4 == 1:
                                proj_q(ht, mu // 4)
                        # oh of the previous window
                        if w in (1, 2, 3, 4, 5, 6):
                            oh_group(w - 1, mu, prev_ex)
                        elif w == 7:
                            if mu < 4:
                                oh_group(6, 2 * mu, prev_ex)
                                oh_group(6, 2 * mu + 1, prev_ex)
                            else:
                                emit_out_group(mu - 4)
                    prev_ex = ex_tiles

            # === tail: window 7 oh + nb1 output projections ===
            for g in range(8):
                oh_group(7, g, prev_ex)
            for nt in range(4, 8):
                emit_out_group(nt)

    nc.compile()
    return nc


def kernel(query, key, value, query_kernel, key_kernel, value_kernel,
           projection_kernel, q_bias, k_bias, v_bias, projection_bias):
    query = np.ascontiguousarray(np.asarray(query, dtype=np.float32))
    key = np.ascontiguousarray(np.asarray(key, dtype=np.float32))
    value = np.ascontiguousarray(np.asarray(value, dtype=np.float32))
    scale = np.float32(1.0 / 8.0)  # 1/sqrt(DH)

    wq = np.ascontiguousarray(
        (np.asarray(query_kernel, np.float32) * scale).transpose(1, 0, 2).reshape(DM, HDH))
    wk = np.ascontiguousarray(
        np.asarray(key_kernel, np.float32).transpose(1, 0, 2).reshape(DM, HDH))
    wv = np.ascontiguousarray(
        np.asarray(value_kernel, np.float32).transpose(1, 0, 2).reshape(DM, HDH))
    wp = np.ascontiguousarray(
        np.asarray(projection_kernel, np.float32).reshape(HDH, DO)
    ).astype(ml_dtypes.bfloat16)
    qb = np.ascontiguousarray(
        (np.asarray(q_bias, np.float32) * scale).reshape(HDH).reshape(4, P).T)
    kb = np.ascontiguousarray(np.asarray(k_bias, np.float32).reshape(HDH).reshape(4, P).T)
    vbrow = np.ascontiguousarray(np.asarray(v_bias, np.float32).reshape(1, HDH))
    pb = np.ascontiguousarray(np.asarray(projection_bias, np.float32).reshape(1, DO))
    identb = np.eye(P, dtype=ml_dtypes.bfloat16)
    ident = np.eye(P, dtype=np.float32)
    ones = np.ones((P, P), dtype=np.float32)

    if "nc" not in _CACHED:
        _CACHED["nc"] = _build()
    nc = _CACHED["nc"]

    shared = dict(wq=wq, wk=wk, wv=wv, wp=wp, qb=qb, kb=kb, vbrow=vbrow, pb=pb,
                  identb=identb, ident=ident, ones=ones)
    in_maps = []
    for c in range(8):
        b, half = c // 2, c % 2
        in_maps.append(dict(
            q=np.ascontiguousarray(query[b, half * NB:(half + 1) * NB, :]),
            k=key[b], v=value[b], **shared))

    trace = os.environ.get("KERNEL_TRACE", "0") == "1"
    try:
        res = run_bass_kernel_spmd(nc, in_maps, core_ids=list(range(8)), trace=trace)
    except ModuleNotFoundError:
        res = run_bass_kernel_spmd(nc, in_maps, core_ids=list(range(8)), trace=False)
    global LAST_EXEC_NS
    LAST_EXEC_NS = res.exec_time_ns
    if trace and res.exec_time_ns is not None:
        print(f"HW exec time: {res.exec_time_ns} ns")
        if res.instructions_and_trace is not None:
            print(f"trace: {res.instructions_and_trace[1]}")

    B = query.shape[0]
    out = np.empty((B, 2 * NB, DO), dtype=np.float32)
    for c in range(8):
        b, half = c // 2, c % 2
        out[b, half * NB:(half + 1) * NB, :] = res.results[c]["out"]
    return out
